# revision 3
# baseline (speedup 1.0000x reference)
"""MGLSTM (AGNN message passing + graph LSTM) on 8 Trainium2 NeuronCores.

Strategy (graph-parallel by destination node, per sharding hint):
  - Host permutes nodes into 8*49 windows of 128 (degree-balanced), pads
    N=50000 -> 50176 = 8 * 6272 rows. Core c owns rows [c*6272, (c+1)*6272).
  - Edges (plus self-loops) are bucketed by dst window, sorted by src bank
    (int16 gather indices address at most 32768 rows per bank), padded to a
    uniform per-window tile count so one SPMD program serves all cores.
  - Node-feature tables have 512B rows: [feat(96) | 1.0 | inv_norm | pad].
    T_h is host-staged (h is an input); T_x / T_n are built on device and
    AllGather'ed (overlapped with the edge passes).
  - Per 128-edge tile: batched dma_gather of src rows (full table) and dst
    rows (local table); fused scalar_tensor_tensor dot -> raw alpha;
    alpha = beta * raw * inv_s * inv_d; s = exp(alpha)  (softmax shift is
    unnecessary: |alpha| <= |beta|, so exp never overflows; the softmax is
    shift-invariant so this matches the reference's max-subtracted form);
    Ms = (iota == dstcol) * s (one tensor_scalar op); PSUM-accumulated
    scatter matmul  acc[128 dst, 97] += Ms.T @ [feat | 1]  gives both the
    weighted numerator and the denominator.
  - betas are read on host (known at trace time) and baked in as immediates.
    The reference's 8 gate AGNNs collapse to 2 when betas[2::2] and
    betas[3::2] are constant (true for setup_inputs); otherwise a numpy
    fallback runs.
"""

import numpy as np
from contextlib import ExitStack

P = 128
H = 96
ROWW = 128           # table row width in fp32 (512B)
BANK = 32768         # int16 index bank
NCORES = 8

_CACHE = {}


# --------------------------------------------------------------------------
# host-side helpers
# --------------------------------------------------------------------------

def _wrap_idx_int16(idx):
    """[n] -> [128, n/16] int16 wrapped layout (idx i at [i%16, i//16]),
    replicated across the 8 groups of 16 partitions."""
    n = idx.shape[0]
    a = np.zeros((16, n // 16), np.int16)
    a[np.arange(n) % 16, np.arange(n) // 16] = idx.astype(np.int16)
    return np.tile(a, (8, 1))


def _np_reference(x, edge_index, h, c, W_in, b_in, Wb1, bb1, Wb2, bb2, betas):
    """Exact numpy port of the reference (safety net for unexpected betas)."""
    EPS = 1e-12
    n = x.shape[0]
    loops = np.arange(n, dtype=edge_index.dtype)
    src = np.concatenate([edge_index[0], loops])
    dst = np.concatenate([edge_index[1], loops])

    def agnn(xf, beta):
        norm = np.sqrt((xf * xf).sum(-1, keepdims=True))
        xn = xf / np.maximum(norm, EPS)
        alpha = beta * (xn[src] * xn[dst]).sum(-1)
        amax = np.full(n, -np.inf, np.float32)
        np.maximum.at(amax, dst, alpha)
        ex = np.exp(alpha - amax[dst])
        den = np.zeros(n, np.float32)
        np.add.at(den, dst, ex)
        w = ex / den[dst]
        out = np.zeros_like(xf)
        np.add.at(out, dst, xf[src] * w[:, None])
        return out

    xt = x @ W_in.T + b_in
    h_N1 = agnn(h, betas[1])
    bg = np.tanh(h @ Wb1.T + bb1 + h_N1 @ Wb2.T + bb2)
    h_N = h + bg

    def sig(v):
        return 1.0 / (1.0 + np.exp(-v))

    f = sig(agnn(xt, betas[2]) + agnn(h_N, betas[3]))
    i = sig(agnn(xt, betas[4]) + agnn(h_N, betas[5]))
    ct = np.tanh(agnn(xt, betas[6]) + agnn(h_N, betas[7]))
    c_new = f * c + i * ct
    o = sig(agnn(xt, betas[8]) + agnn(h_N, betas[9]))
    h_new = o * np.tanh(c_new)
    return h_new, c_new


def _make_table(feat, inv):
    """[n, 96] + [n] -> [n, 128] fp32 rows [feat | 1.0 | inv | 0...]."""
    n = feat.shape[0]
    t = np.zeros((n, ROWW), np.float32)
    t[:, 0:H] = feat
    t[:, H] = 1.0
    t[:, H + 1] = inv
    return t


def _prep(x, edge_index, h, c):
    """Node permutation, edge bucketing, uniform padding, index arrays."""
    N = x.shape[0]
    NPC = -(-N // (NCORES * P)) * P  # rows per core, multiple of 128
    WPC = NPC // P
    NPAD = NPC * NCORES
    NW = NPAD // P

    src0 = edge_index[0].astype(np.int64)
    dst0 = edge_index[1].astype(np.int64)
    deg = np.bincount(dst0, minlength=N) + 1  # incl self-loop

    # degree-balanced assignment of nodes to the NW windows
    order = np.argsort(-deg, kind="stable")
    wcap = np.full(NW, P, np.int64)
    # reserve virtual slots evenly: last windows of each core lose slots
    nvirt = NPAD - N
    vper = nvirt // NCORES
    for cc in range(NCORES):
        wleft = vper
        wi = (cc + 1) * WPC - 1
        while wleft > 0:
            take = min(wleft, P - 1)  # keep >=1 real node per window
            wcap[wi] -= take
            wleft -= take
            wi -= 1
    rem = nvirt - vper * NCORES
    wi = NW - 1
    while rem > 0:
        take = min(rem, wcap[wi] - 1)
        wcap[wi] -= take
        rem -= take
        wi -= 1
    assert wcap.sum() == N

    # greedy: heaviest nodes to the currently lightest window with capacity
    wsum = np.zeros(NW, np.int64)
    wfill = np.zeros(NW, np.int64)
    row_of = np.empty(N, np.int64)
    import heapq

    heap = [(0, int(w)) for w in range(NW)]
    heapq.heapify(heap)
    for nd in order:
        while True:
            s, w = heapq.heappop(heap)
            if wfill[w] < wcap[w]:
                break
        row_of[nd] = w * P + wfill[w]
        wfill[w] += 1
        wsum[w] += deg[nd]
        heapq.heappush(heap, (int(wsum[w]), w))

    node_of_row = np.full(NPAD, -1, np.int64)
    node_of_row[row_of] = np.arange(N)

    srcr = row_of[src0]
    dstr = row_of[dst0]
    # self loops on real rows
    real_rows = row_of  # rows of real nodes
    srcr = np.concatenate([srcr, real_rows])
    dstr = np.concatenate([dstr, real_rows])

    w_of_e = dstr // P
    o = np.argsort(w_of_e * 2 + (srcr >= BANK), kind="stable")
    srcr, dstr, w_of_e = srcr[o], dstr[o], w_of_e[o]
    bank = (srcr >= BANK).astype(np.int64)
    # per-window bank counts
    cnt0 = np.bincount(w_of_e, weights=1 - bank, minlength=NW).astype(np.int64)
    cnt1 = np.bincount(w_of_e, weights=bank, minlength=NW).astype(np.int64)
    T0 = max(1, int(-(-cnt0.max() // P)))
    T1 = max(1, int(-(-cnt1.max() // P)))
    TT = T0 + T1

    starts = np.concatenate([[0], np.cumsum(cnt0 + cnt1)]).astype(np.int64)

    # uniform padded per-window edge slots
    idx_b0 = np.zeros((NCORES, WPC, T0 * P), np.int64)
    idx_b1 = np.zeros((NCORES, WPC, T1 * P), np.int64)
    idx_d = np.zeros((NCORES, WPC, TT * P), np.int64)
    dstcol = np.full((NCORES, WPC, TT * P), -1.0, np.float32)
    for w in range(NW):
        cc, wl = divmod(w, WPC)
        s0, n0, n1 = starts[w], int(cnt0[w]), int(cnt1[w])
        sw = srcr[s0 : s0 + n0 + n1]
        dw = dstr[s0 : s0 + n0 + n1]
        idx_b0[cc, wl, :n0] = sw[:n0]
        idx_b1[cc, wl, :n1] = sw[n0:] - BANK
        # edge slot positions: bank0 tiles then bank1 tiles
        pos0 = np.arange(n0)
        pos1 = T0 * P + np.arange(n1)
        pos = np.concatenate([pos0, pos1])
        idx_d[cc, wl, pos] = dw - cc * NPC
        dstcol[cc, wl, pos] = (dw % P).astype(np.float32)

    def wrapped(arr, ntile):
        # [NCORES, WPC, ntile*P] -> [NCORES, 128, WPC*ntile*8] int16
        out = np.empty((NCORES, P, arr.shape[1] * ntile * 8), np.int16)
        for cc in range(NCORES):
            cols = [_wrap_idx_int16(arr[cc, wl]) for wl in range(arr.shape[1])]
            out[cc] = np.concatenate(cols, axis=1)
        return out

    dstcol_dev = dstcol.reshape(NCORES, WPC, TT, P).transpose(0, 3, 1, 2).reshape(
        NCORES, P, WPC * TT
    )
    return dict(
        NPC=NPC, WPC=WPC, NPAD=NPAD, T0=T0, T1=T1, TT=TT,
        row_of=row_of, node_of_row=node_of_row,
        idx_b0=wrapped(idx_b0, T0), idx_b1=wrapped(idx_b1, T1),
        idx_d=wrapped(idx_d, TT), dstcol=np.ascontiguousarray(dstcol_dev),
    )


# --------------------------------------------------------------------------
# device program
# --------------------------------------------------------------------------

def _build_program(pp, betas):
    import concourse.bacc as bacc
    import concourse.mybir as mybir
    import concourse.tile as tile
    from concourse import library_config
    from concourse.masks import make_identity

    NPC, WPC, NPAD = pp["NPC"], pp["WPC"], pp["NPAD"]
    T0, T1, TT = pp["T0"], pp["T1"], pp["TT"]
    b1, b2, b3 = float(betas[1]), float(betas[2]), float(betas[3])
    AL = mybir.AluOpType
    AF = mybir.ActivationFunctionType
    f32 = mybir.dt.float32

    nc = bacc.Bacc("TRN2", target_bir_lowering=False, debug=False,
                   num_devices=NCORES)

    # inputs
    th_full = nc.dram_tensor("th_full", [NPAD, ROWW], f32, kind="ExternalInput")
    th_loc = nc.dram_tensor("th_loc", [NPC, ROWW], f32, kind="ExternalInput")
    xTc = nc.dram_tensor("xTc", [2, P, NPC], f32, kind="ExternalInput")
    hT_in = nc.dram_tensor("hT", [H, NPC], f32, kind="ExternalInput")
    c_in = nc.dram_tensor("c_arr", [P, WPC * H], f32, kind="ExternalInput")
    ib0_in = nc.dram_tensor("idx_b0", [P, WPC * T0 * 8], mybir.dt.int16, kind="ExternalInput")
    ib1_in = nc.dram_tensor("idx_b1", [P, WPC * T1 * 8], mybir.dt.int16, kind="ExternalInput")
    ibd_in = nc.dram_tensor("idx_d", [P, WPC * TT * 8], mybir.dt.int16, kind="ExternalInput")
    dcol_in = nc.dram_tensor("dstcol", [P, WPC * TT], f32, kind="ExternalInput")
    winT_in = nc.dram_tensor("W_inT", [2, P, H], f32, kind="ExternalInput")
    wb1T_in = nc.dram_tensor("Wb1T", [H, H], f32, kind="ExternalInput")
    wb2T_in = nc.dram_tensor("Wb2T", [H, H], f32, kind="ExternalInput")
    bin_in = nc.dram_tensor("b_in", [H, 1], f32, kind="ExternalInput")
    bb_in = nc.dram_tensor("bb", [H, 1], f32, kind="ExternalInput")
    iota_in = nc.dram_tensor("iota", [P, P], f32, kind="ExternalInput")

    out_h = nc.dram_tensor("out_h", [NPC, H], f32, kind="ExternalOutput")
    out_c = nc.dram_tensor("out_c", [NPC, H], f32, kind="ExternalOutput")

    # internal DRAM
    tx_shard = nc.dram_tensor("tx_shard", [NPC, ROWW], f32, kind="Internal")
    tn_shard = nc.dram_tensor("tn_shard", [NPC, ROWW], f32, kind="Internal")
    tx_full = nc.dram_tensor("tx_full", [NPAD, ROWW], f32, kind="Internal", addr_space="Shared")
    tn_full = nc.dram_tensor("tn_full", [NPAD, ROWW], f32, kind="Internal", addr_space="Shared")
    tx_stage = nc.dram_tensor("tx_stage", [NPC, ROWW], f32, kind="Internal")
    tn_stage = nc.dram_tensor("tn_stage", [NPC, ROWW], f32, kind="Internal")

    rg = [list(range(NCORES))]

    with tile.TileContext(nc) as tc, ExitStack() as ctx:
        sb = ctx.enter_context(tc.tile_pool(name="sb", bufs=1))
        sb2 = ctx.enter_context(tc.tile_pool(name="sb2", bufs=2))
        sb3 = ctx.enter_context(tc.tile_pool(name="sb3", bufs=3))
        ps = ctx.enter_context(tc.tile_pool(name="ps", bufs=2, space="PSUM"))

        nc.gpsimd.load_library(library_config.mlp)

        # ---- persistent SBUF ----
        ib0_sb = sb.tile([P, WPC * T0 * 8], mybir.dt.int16)
        nc.sync.dma_start(ib0_sb[:], ib0_in[:])
        ib1_sb = sb.tile([P, WPC * T1 * 8], mybir.dt.int16)
        nc.sync.dma_start(ib1_sb[:], ib1_in[:])
        ibd_sb = sb.tile([P, WPC * TT * 8], mybir.dt.int16)
        nc.sync.dma_start(ibd_sb[:], ibd_in[:])
        dcol_sb = sb.tile([P, WPC * TT], f32)
        nc.sync.dma_start(dcol_sb[:], dcol_in[:])
        iota_sb = sb.tile([P, P], f32)
        nc.sync.dma_start(iota_sb[:], iota_in[:])
        idm = sb.tile([P, P], f32)
        make_identity(nc, idm[:])
        hT_sb = sb.tile([H, NPC], f32)
        nc.sync.dma_start(hT_sb[:], hT_in[:])
        c_sb = sb.tile([P, WPC * H], f32)
        nc.sync.dma_start(c_sb[:], c_in[:])
        winT_sb = sb.tile([P, 2, H], f32)
        nc.sync.dma_start(winT_sb[:], winT_in.ap().rearrange("c p h -> p c h"))
        wb1T_sb = sb.tile([H, H], f32)
        nc.sync.dma_start(wb1T_sb[:], wb1T_in[:])
        wb2T_sb = sb.tile([H, H], f32)
        nc.sync.dma_start(wb2T_sb[:], wb2T_in[:])
        bin_sb = sb.tile([H, 1], f32)
        nc.sync.dma_start(bin_sb[:], bin_in[:])
        bb_sb = sb.tile([H, 1], f32)
        nc.sync.dma_start(bb_sb[:], bb_in[:])

        hn1T_sb = sb.tile([H, NPC], f32)      # AGNN(h) transposed
        sx_sb = sb.tile([P, WPC * H], f32)    # AGNN(xt) gate pre-act rows

        # =========== helper: normalize strip -> inv_norm strip ===========
        def inv_chain(norm2_strip, inv_strip, tag):
            nmax = sb.tile([P, WPC], f32, tag=f"nmax{tag}")
            nc.vector.tensor_scalar(out=nmax[:], in0=norm2_strip[:], scalar1=1e-24,
                                    scalar2=None, op0=AL.max)
            sq = sb.tile([P, WPC], f32, tag=f"sq{tag}")
            nc.scalar.activation(sq[:], nmax[:], AF.Sqrt)
            r0 = sb.tile([P, WPC], f32, tag=f"r0{tag}")
            nc.vector.reciprocal(r0[:], sq[:])
            # one Newton step: inv = r0 * (1.5 - 0.5 * nmax * r0^2)
            y2 = sb.tile([P, WPC], f32, tag=f"y2{tag}")
            nc.vector.tensor_tensor(out=y2[:], in0=r0[:], in1=r0[:], op=AL.mult)
            t = sb.tile([P, WPC], f32, tag=f"t{tag}")
            nc.vector.tensor_tensor(out=t[:], in0=nmax[:], in1=y2[:], op=AL.mult)
            nc.vector.tensor_scalar(out=t[:], in0=t[:], scalar1=-0.5, scalar2=1.5,
                                    op0=AL.mult, op1=AL.add)
            nc.vector.tensor_tensor(out=inv_strip[:], in0=r0[:], in1=t[:], op=AL.mult)

        # =========== phase N1: xt rows + T_x table ===========
        n2x = sb.tile([P, WPC], f32)
        invx = sb.tile([P, WPC], f32)
        for w in range(WPC):
            xt_ch = sb2.tile([P, 2, P], f32, tag="xt_ch")
            nc.sync.dma_start(xt_ch[:], xTc.ap()[:, :, w * P:(w + 1) * P].rearrange("c p r -> p c r"))
            xtT_ps = ps.tile([H, P], f32, space="PSUM", tag="tp")
            for cch in range(2):
                nc.tensor.matmul(xtT_ps[:], lhsT=winT_sb[:, cch, :], rhs=xt_ch[:, cch, :],
                                 start=(cch == 0), stop=(cch == 1))
            xtT_sb = sb2.tile([H, P], f32, tag="xtT_sb")
            nc.scalar.activation(xtT_sb[:], xtT_ps[:], AF.Identity, bias=bin_sb[:])
            row_ps = ps.tile([P, H], f32, space="PSUM", tag="tp")
            nc.tensor.transpose(row_ps[:], xtT_sb[:], idm[0:H, 0:H])
            rowb = sb2.tile([P, ROWW], f32, tag="rowb")
            nc.vector.tensor_copy(out=rowb[:, 0:H], in_=row_ps[:])
            nc.vector.memset(rowb[:, H:ROWW], 0.0)
            nc.vector.memset(rowb[:, H:H + 1], 1.0)
            scr = sb2.tile([P, H], f32, tag="scrT")
            nc.vector.scalar_tensor_tensor(out=scr[:], in0=rowb[:, 0:H], scalar=1.0,
                                           in1=rowb[:, 0:H], op0=AL.mult, op1=AL.mult,
                                           accum_out=n2x[:, w:w + 1])
            nc.sync.dma_start(tx_stage[w * P:(w + 1) * P, :], rowb[:])
        inv_chain(n2x, invx, "x")
        for w in range(WPC):
            rowb2 = sb2.tile([P, ROWW], f32, tag="rowb2")
            nc.sync.dma_start(rowb2[:], tx_stage[w * P:(w + 1) * P, :])
            nc.vector.tensor_copy(out=rowb2[:, H + 1:H + 2], in_=invx[:, w:w + 1])
            nc.sync.dma_start(tx_shard[w * P:(w + 1) * P, :], rowb2[:])
        nc.gpsimd.collective_compute("AllGather", AL.bypass, replica_groups=rg,
                                     ins=[tx_shard[:]], outs=[tx_full[:]])

        # =========== edge pass helper ===========
        def edge_pass(full_tbl, loc_tbl, beta, out_cb, tagp):
            for w in range(WPC):
                gsrc = sb2.tile([P, TT, ROWW], f32, tag="gsrc")
                if T0:
                    nc.gpsimd.dma_gather(
                        gsrc[:, 0:T0, :], full_tbl[:],
                        ib0_sb[:, w * T0 * 8:(w + 1) * T0 * 8],
                        T0 * P, T0 * P, ROWW, elem_step=ROWW, single_packet=False)
                if T1:
                    b1base = full_tbl[BANK:, :] if NPAD > BANK else full_tbl[:]
                    nc.gpsimd.dma_gather(
                        gsrc[:, T0:TT, :], b1base,
                        ib1_sb[:, w * T1 * 8:(w + 1) * T1 * 8],
                        T1 * P, T1 * P, ROWW, elem_step=ROWW, single_packet=False)
                gdst = sb2.tile([P, TT, ROWW], f32, tag="gdst")
                nc.gpsimd.dma_gather(
                    gdst[:], loc_tbl[:],
                    ibd_sb[:, w * TT * 8:(w + 1) * TT * 8],
                    TT * P, TT * P, ROWW, elem_step=ROWW, single_packet=False)

                araw = sb2.tile([P, TT], f32, tag="araw")
                scr = sb2.tile([P, H], f32, tag="scr")
                for t in range(TT):
                    nc.vector.scalar_tensor_tensor(
                        out=scr[:], in0=gsrc[:, t, 0:H], scalar=beta,
                        in1=gdst[:, t, 0:H], op0=AL.mult, op1=AL.mult,
                        accum_out=araw[:, t:t + 1])
                nc.vector.tensor_tensor(out=araw[:], in0=araw[:],
                                        in1=gsrc[:, :, H + 1], op=AL.mult)
                nc.vector.tensor_tensor(out=araw[:], in0=araw[:],
                                        in1=gdst[:, :, H + 1], op=AL.mult)
                s_strip = sb2.tile([P, TT], f32, tag="s")
                nc.scalar.activation(s_strip[:], araw[:], AF.Exp)

                acc_ps = ps.tile([P, H + 1], f32, space="PSUM", tag="acc")
                ms = sb2.tile([P, P], f32, tag="ms")
                for t in range(TT):
                    nc.vector.tensor_scalar(
                        out=ms[:], in0=iota_sb[:],
                        scalar1=dcol_sb[:, w * TT + t:w * TT + t + 1],
                        scalar2=s_strip[:, t:t + 1],
                        op0=AL.is_equal, op1=AL.mult)
                    nc.tensor.matmul(acc_ps[:], lhsT=ms[:], rhs=gsrc[:, t, 0:H + 1],
                                     start=(t == 0), stop=(t == TT - 1))
                den = sb2.tile([P, 1], f32, tag="den")
                nc.vector.tensor_scalar(out=den[:], in0=acc_ps[:, H:H + 1],
                                        scalar1=1e-30, scalar2=None, op0=AL.add)
                rec = sb2.tile([P, 1], f32, tag="rec")
                nc.vector.reciprocal(rec[:], den[:])
                rows = sb2.tile([P, H], f32, tag="rows")
                nc.vector.tensor_scalar(out=rows[:], in0=acc_ps[:, 0:H],
                                        scalar1=rec[:], scalar2=None, op0=AL.mult)
                out_cb(w, rows)

        # =========== E1: AGNN(h) -> hn1T strip ===========
        def e1_out(w, rows):
            t_ps = ps.tile([H, P], f32, space="PSUM", tag="tp")
            nc.tensor.transpose(t_ps[:], rows[:], idm[:])
            nc.vector.tensor_copy(out=hn1T_sb[:, w * P:(w + 1) * P], in_=t_ps[:])

        edge_pass(th_full, th_loc, b1, e1_out, "e1")

        # =========== G: beta_gate, h_N, T_n table ===========
        n2n = sb.tile([P, WPC], f32)
        invn = sb.tile([P, WPC], f32)
        for w in range(WPC):
            bg_ps = ps.tile([H, P], f32, space="PSUM", tag="tp")
            nc.tensor.matmul(bg_ps[:], lhsT=wb1T_sb[:], rhs=hT_sb[:, w * P:(w + 1) * P],
                             start=True, stop=False)
            nc.tensor.matmul(bg_ps[:], lhsT=wb2T_sb[:], rhs=hn1T_sb[:, w * P:(w + 1) * P],
                             start=False, stop=True)
            bgT = sb2.tile([H, P], f32, tag="bgT")
            nc.scalar.activation(bgT[:], bg_ps[:], AF.Tanh, bias=bb_sb[:])
            hnT = sb2.tile([H, P], f32, tag="hnT")
            nc.vector.tensor_tensor(out=hnT[:], in0=hT_sb[:, w * P:(w + 1) * P],
                                    in1=bgT[:], op=AL.add)
            row_ps = ps.tile([P, H], f32, space="PSUM", tag="tp")
            nc.tensor.transpose(row_ps[:], hnT[:], idm[0:H, 0:H])
            rowb = sb2.tile([P, ROWW], f32, tag="rowb")
            nc.vector.tensor_copy(out=rowb[:, 0:H], in_=row_ps[:])
            nc.vector.memset(rowb[:, H:ROWW], 0.0)
            nc.vector.memset(rowb[:, H:H + 1], 1.0)
            scr = sb2.tile([P, H], f32, tag="scrT")
            nc.vector.scalar_tensor_tensor(out=scr[:], in0=rowb[:, 0:H], scalar=1.0,
                                           in1=rowb[:, 0:H], op0=AL.mult, op1=AL.mult,
                                           accum_out=n2n[:, w:w + 1])
            nc.sync.dma_start(tn_stage[w * P:(w + 1) * P, :], rowb[:])
        inv_chain(n2n, invn, "n")
        for w in range(WPC):
            rowb2 = sb2.tile([P, ROWW], f32, tag="rowb2")
            nc.sync.dma_start(rowb2[:], tn_stage[w * P:(w + 1) * P, :])
            nc.vector.tensor_copy(out=rowb2[:, H + 1:H + 2], in_=invn[:, w:w + 1])
            nc.sync.dma_start(tn_shard[w * P:(w + 1) * P, :], rowb2[:])
        nc.gpsimd.collective_compute("AllGather", AL.bypass, replica_groups=rg,
                                     ins=[tn_shard[:]], outs=[tn_full[:]])

        # =========== E2a: AGNN(xt) ===========
        def e2a_out(w, rows):
            nc.vector.tensor_copy(out=sx_sb[:, w * H:(w + 1) * H], in_=rows[:])

        edge_pass(tx_full, tx_shard, b2, e2a_out, "e2a")

        # =========== E2b: AGNN(h_N) + LSTM tail ===========
        def e2b_out(w, rows):
            s = sb2.tile([P, H], f32, tag="s_gate")
            nc.vector.tensor_tensor(out=s[:], in0=sx_sb[:, w * H:(w + 1) * H],
                                    in1=rows[:], op=AL.add)
            sig = sb2.tile([P, H], f32, tag="sig")
            nc.scalar.activation(sig[:], s[:], AF.Sigmoid)
            th = sb2.tile([P, H], f32, tag="th")
            nc.scalar.activation(th[:], s[:], AF.Tanh)
            cpt = sb2.tile([P, H], f32, tag="cpt")
            nc.vector.tensor_tensor(out=cpt[:], in0=c_sb[:, w * H:(w + 1) * H],
                                    in1=th[:], op=AL.add)
            cn = sb2.tile([P, H], f32, tag="cn")
            nc.vector.tensor_tensor(out=cn[:], in0=sig[:], in1=cpt[:], op=AL.mult)
            tcn = sb2.tile([P, H], f32, tag="tcn")
            nc.scalar.activation(tcn[:], cn[:], AF.Tanh)
            hn = sb2.tile([P, H], f32, tag="hn_out")
            nc.vector.tensor_tensor(out=hn[:], in0=sig[:], in1=tcn[:], op=AL.mult)
            nc.sync.dma_start(out_c[w * P:(w + 1) * P, :], cn[:])
            nc.sync.dma_start(out_h[w * P:(w + 1) * P, :], hn[:])

        edge_pass(tn_full, tn_shard, b3, e2b_out, "e2b")

    nc.compile()
    return nc


# --------------------------------------------------------------------------
# entry point
# --------------------------------------------------------------------------

def kernel(x, edge_index, h, c, W_in, b_in, Wg1, bg1, Wg2, bg2, Wb1, bb1,
           Wb2, bb2, betas):
    x = np.asarray(x, np.float32)
    edge_index = np.asarray(edge_index)
    h = np.asarray(h, np.float32)
    c = np.asarray(c, np.float32)
    betas_np = np.asarray(betas, np.float32)

    uniform = (np.all(betas_np[2::2] == betas_np[2]) and
               np.all(betas_np[3::2] == betas_np[3]))
    if not uniform:
        return _np_reference(x, edge_index, h, c,
                             np.asarray(W_in, np.float32), np.asarray(b_in, np.float32),
                             np.asarray(Wb1, np.float32), np.asarray(bb1, np.float32),
                             np.asarray(Wb2, np.float32), np.asarray(bb2, np.float32),
                             betas_np)

    from concourse.bass_utils import run_bass_kernel_spmd

    N = x.shape[0]
    pp = _prep(x, edge_index, h, c)
    NPC, NPAD, WPC = pp["NPC"], pp["NPAD"], pp["WPC"]
    row_of, node_of_row = pp["row_of"], pp["node_of_row"]

    key = (N, x.shape[1], edge_index.shape[1], pp["T0"], pp["T1"],
           float(betas_np[1]), float(betas_np[2]), float(betas_np[3]))
    if key not in _CACHE:
        _CACHE[key] = _build_program(pp, betas_np)
    nc = _CACHE[key]

    # host staging (permuted row order)
    hp = np.zeros((NPAD, H), np.float32)
    hp[row_of] = h
    norm_h = np.sqrt((hp.astype(np.float64) ** 2).sum(-1))
    inv_h = (1.0 / np.maximum(norm_h, 1e-12)).astype(np.float32)
    inv_h[node_of_row < 0] = 0.0
    th_full = _make_table(hp, inv_h)

    xp = np.zeros((NPAD, x.shape[1]), np.float32)
    xp[row_of] = x
    cp = np.zeros((NPAD, H), np.float32)
    cp[row_of] = c

    W_inT2 = np.ascontiguousarray(
        np.asarray(W_in, np.float32).T.reshape(2, P, H))
    iota = np.tile(np.arange(P, dtype=np.float32)[None, :], (P, 1))
    bb = (np.asarray(bb1, np.float32) + np.asarray(bb2, np.float32)).reshape(H, 1)

    in_maps = []
    for cc in range(NCORES):
        lo, hi = cc * NPC, (cc + 1) * NPC
        in_maps.append(dict(
            th_full=th_full,
            th_loc=np.ascontiguousarray(th_full[lo:hi]),
            xTc=np.ascontiguousarray(xp[lo:hi].T.reshape(2, P, NPC)),
            hT=np.ascontiguousarray(hp[lo:hi].T),
            c_arr=np.ascontiguousarray(
                cp[lo:hi].reshape(WPC, P, H).transpose(1, 0, 2).reshape(P, WPC * H)),
            idx_b0=pp["idx_b0"][cc], idx_b1=pp["idx_b1"][cc],
            idx_d=pp["idx_d"][cc], dstcol=pp["dstcol"][cc],
            W_inT=W_inT2,
            Wb1T=np.ascontiguousarray(np.asarray(Wb1, np.float32).T),
            Wb2T=np.ascontiguousarray(np.asarray(Wb2, np.float32).T),
            b_in=np.asarray(b_in, np.float32).reshape(H, 1),
            bb=bb,
            iota=iota,
        ))

    res = run_bass_kernel_spmd(nc, in_maps, core_ids=list(range(NCORES)))

    h_new = np.empty((NPAD, H), np.float32)
    c_new = np.empty((NPAD, H), np.float32)
    for cc in range(NCORES):
        h_new[cc * NPC:(cc + 1) * NPC] = res.results[cc]["out_h"]
        c_new[cc * NPC:(cc + 1) * NPC] = res.results[cc]["out_c"]
    return h_new[row_of], c_new[row_of]
